# revision 70
# baseline (speedup 1.0000x reference)
"""GQA attention (B=4, T=2048, Hq=16, Hkv=4, hd=128, D=2048) on 8 trn2 cores.

Sharding: core c = (batch b = c//2, row-parity r = c%2). Each core computes
attention for batch b on query row-tiles {2t + r : t in 0..7} (interleaved
128-row tiles, which balances causal work across the two cores of a batch)
and the full output projection for those rows. K/V are computed for the full
sequence on both cores of a batch (cheap), so no cross-core communication is
needed; the host just concatenates disjoint output row slices.

Per-core kernel layouts (everything transposed so no on-device transposes):
  QT/KT:  [head_dim, tok]  (from matmul(lhsT=w_T_tile, rhs=hidden_T_tile))
  V:      [tok, head_dim]  (natural)
  S^T:    [k, q] = KT_tile.T @ QT  -> exp -> P^T
  attn^T: [d, q] = V.T @ P^T       (lhsT=V, rhs=P^T)
  out:    [tok, D] = attn^T.T @ woT
RoPE rotate_half is a signed 128x128 permutation applied with one bf16
matmul; cos/sin arrive pre-transposed (and pre-scaled by 1/sqrt(128) on the
Q side). Softmax skips max-subtraction (scores are O(10) here). Causal
masking is multiplicative {0,1} on exp(S^T) at the diagonal junction only.

Softmax denominators: per pair of k-tiles the two exp'd P^T tiles are summed
on DVE (bf16) and a single ones-column matmul accumulates the pair into a
[1, 512] PSUM row per (head, q-half) -- half the PE columns of per-tile ones
matmuls. Normalization is per-group: ACT reciprocal of the PSUM row (bf16),
PE broadcast matmul (ones-row x recip) into PSUM, and one DVE multiply
produces the normalized ATT slice. No DRAM round-trip, no end-of-phase
stall; phase-4 weights prefetch during phase 3 and phase-2 inputs during
phase 1.
"""

import numpy as np
import ml_dtypes

import concourse.bass as bass
import concourse.mybir as mybir
import concourse.tile as tile
from concourse import bacc
from concourse.bass_utils import run_bass_kernel_spmd

F32 = mybir.dt.float32
BF16 = mybir.dt.bfloat16
AF = mybir.ActivationFunctionType
NPBF16 = ml_dtypes.bfloat16

P = 128      # partitions / head_dim / row-tile
T = 2048     # full seq len per batch
TQ = 1024    # query rows per core
NH = 16      # query heads
NKV = 4      # kv heads
D = 2048     # model dim
DT = D // P  # 16 D-tiles
N_CORES = 8


def build_program(phases=(1, 2, 3, 4), rep=1):
    nc = bacc.Bacc(
        "TRN2", target_bir_lowering=False, debug=False, enable_asserts=False
    )

    def din(name, shape, dt=BF16):
        return nc.dram_tensor(name, shape, dt, kind="ExternalInput").ap()

    ht = din("hth", [D, TQ])          # own token-half hidden.T (for K/V proj)
    hq = din("hq", [D, TQ])           # own-rows hidden.T    (for Q proj)
    wqt = din("wqt", [D, NH * P])     # wq.T
    wkt = din("wkt", [D, NKV * P])    # wk.T
    wvt = din("wvt", [D, NKV * P])    # wv.T
    wot = din("wot", [NH * P, D])     # wo.T
    cq = din("cq", [P, TQ], F32)      # cos.T own rows, pre-scaled 1/sqrt(128)
    sq = din("sq", [P, TQ], F32)
    ck = din("ck", [P, TQ], F32)      # cos.T own token-half (for K)
    sk = din("sk", [P, TQ], F32)
    maskab = din("maskab", [P, 2, P])  # junction masks for (even j, odd j)
    pm = din("pm", [P, P])            # signed rotate_half permutation (bf16)
    onesc = din("onesc", [P, 1])      # ones column (denominator lhsT)
    selr = din("selr", [NH, 2 * NH * P])  # row-select bcast lhsT
    tick = din("tick", [1, 8], F32)   # timing-chain passthrough

    out = nc.dram_tensor("out", [TQ, D], F32, kind="ExternalOutput").ap()
    tock = nc.dram_tensor("tock", [1, 8], F32, kind="ExternalOutput").ap()
    # K/V pair-exchange staging: each core computes K/V for its token half,
    # the pair AllGathers the halves (cores of a batch are adjacent ids)
    NB = NKV * TQ // 512 + DT // 2        # 8 K-blocks + 8 V-blocks of 512
    kvd = nc.dram_tensor("kvd", [P, NB, 512], BF16, kind="Internal").ap()
    kvg = nc.dram_tensor("kvg", [2, P, NB, 512], BF16, kind="Internal").ap()

    with tile.TileContext(nc) as tc:
        for rp in range(rep):
            _emit(nc, tc, ht, hq, wqt, wkt, wvt, wot, cq, sq, ck, sk,
                  maskab, pm, onesc, selr, tick, out, tock, kvd, kvg,
                  phases=phases, pfx=f"_r{rp}" if rep > 1 else "")
    nc.compile()
    return nc


def _emit(nc, tc, ht, hq, wqt, wkt, wvt, wot, cq, sq, ck, sk,
          maskab, pm, onesc, selr, tick, out, tock, kvd, kvg,
          phases=(1, 2, 3, 4), pfx=""):
    from contextlib import ExitStack

    with ExitStack() as ctx:
        # ---- persistent tiles (live across phases) ----
        pers = ctx.enter_context(tc.tile_pool(name="pers" + pfx, bufs=1))
        KT = pers.tile([P, NKV, T], BF16, tag="KT")     # (d, kvh, k) rope'd
        Vsb = pers.tile([P, DT, NKV * P], BF16, tag="V")  # (k%128, ktile, dv)
        QT = pers.tile([P, NH, TQ], BF16, tag="QT")     # (d, h, q) rope'd+scaled
        pm_sb = pers.tile([P, P], BF16, tag="pm")
        ones_sb = pers.tile([P, 1], BF16, tag="ones")
        mask_sb = pers.tile([P, 2, P], BF16, tag="mask")
        tick_sb = pers.tile([1, 8], F32, tag="tick")

        nc.sync.dma_start(pm_sb[:], pm[:])
        nc.sync.dma_start(ones_sb[:], onesc[:])
        nc.sync.dma_start(mask_sb[:], maskab[:])
        nc.sync.dma_start(tick_sb[:], tick[:])

        # timing-variant support: zero tiles whose producer phase is skipped
        if 1 not in phases and 3 in phases:
            nc.any.memzero(KT[:])
            nc.any.memzero(Vsb[:])
        if 2 not in phases and 3 in phases:
            nc.any.memzero(QT[:])

        # phase-2 pools are created up front so cq/sq, hq and the first wq
        # quarter stream in during phase 1 (removes the phase-1->2 DMA gap);
        # they close after phase 2 to release SBUF for phases 3/4.  The
        # prefetch dma_starts are deferred until after phase 1's own DMAs so
        # they sit behind them in the dispatch queue.
        p2stack = ctx.enter_context(ExitStack())
        p2 = p2stack.enter_context(tc.tile_pool(name="p2" + pfx, bufs=1))
        p2w = p2stack.enter_context(tc.tile_pool(name="p2w" + pfx, bufs=2))
        cq_sb = p2.tile([P, TQ], F32, tag="cq")
        sq_sb = p2.tile([P, TQ], F32, tag="sq")
        hqs = p2.tile([P, DT, TQ], BF16, tag="hq")
        wq_tiles = {}
        hq_r = hq.rearrange("(t p) n -> p t n", p=P)
        wq_r = wqt.rearrange("(t p) n -> p t n", p=P)

        def prefetch_phase2():
            nc.sync.dma_start(cq_sb[:], cq[:])
            nc.sync.dma_start(sq_sb[:], sq[:])
            wq_tiles[0] = p2w.tile([P, DT, 512], BF16, tag="wq", name="wq0")
            nc.sync.dma_start(wq_tiles[0][:], wq_r[:, :, 0:512])
            for q4 in range(0, DT, 4):
                nc.sync.dma_start(hqs[:, q4:q4 + 4], hq_r[:, q4:q4 + 4, :])

        # ================= phase 1: K/V projections + K RoPE =================
        # hidden.T is streamed in 512-token quarters (double-buffered) to fit
        # SBUF; wk/wv stay resident. Per quarter: K proj for 4 kv heads +
        # RoPE, then V proj for its 4 token-tiles. DMA issue order puts the
        # first K matmul's dependencies (wk tile 0, ht tile 0) first.
        if 1 in phases:
          with tc.tile_pool(name="p1" + pfx, bufs=1) as p1, \
             tc.tile_pool(name="p1h" + pfx, bufs=2) as p1h, \
             tc.tile_pool(name="p1t" + pfx, bufs=2) as p1t, \
             tc.tile_pool(name="ps1" + pfx, bufs=1, space="PSUM") as ps1:
            CH = 256             # phase-1 token-chunk width
            NCH = TQ // CH       # own token-half only; pair exchange below
            wks = p1.tile([P, DT, NKV * P], BF16, tag="wk")
            wvs = p1.tile([P, DT, NKV * P], BF16, tag="wv")
            ck_sb = p1.tile([P, TQ], F32, tag="ck")
            sk_sb = p1.tile([P, TQ], F32, tag="sk")
            hts0 = p1h.tile([P, DT, CH], BF16, tag="ht", name="hts0")
            wk_r = wkt.rearrange("(t p) n -> p t n", p=P)
            wv_r = wvt.rearrange("(t p) n -> p t n", p=P)
            ht_r = ht.rearrange("(t p) n -> p t n", p=P)
            for qa, qb in ((0, 1), (1, 2), (2, 4), (4, 8), (8, 12), (12, 16)):
                nc.sync.dma_start(wks[:, qa:qb], wk_r[:, qa:qb, :])
                nc.sync.dma_start(hts0[:, qa:qb], ht_r[:, qa:qb, 0:CH])
                if qb == 4:     # first chunks' rope tables arrive early
                    nc.sync.dma_start(ck_sb[:, 0:512], ck[:, 0:512])
                    nc.sync.dma_start(sk_sb[:, 0:512], sk[:, 0:512])
            nc.sync.dma_start(ck_sb[:, 512:TQ], ck[:, 512:TQ])
            nc.sync.dma_start(sk_sb[:, 512:TQ], sk[:, 512:TQ])
            for q4 in range(0, DT, 4):
                nc.sync.dma_start(wvs[:, q4:q4 + 4], wv_r[:, q4:q4 + 4, :])

            def k_rope(kvh, tsl, ktmp):
                # rotate via Pm matmul (bf16), then combine with cos/sin
                rot = ps1.tile([P, CH], F32, tag="rot", bufs=2)
                nc.tensor.matmul(rot[:], lhsT=pm_sb[:], rhs=ktmp[:],
                                 start=True, stop=True)
                t2 = p1t.tile([P, CH], F32, tag="t2")
                nc.vector.tensor_mul(t2[:], rot[:], sk_sb[:, tsl])
                nc.vector.tensor_mul(KT[:, kvh, tsl], ktmp[:], ck_sb[:, tsl])
                nc.vector.tensor_add(KT[:, kvh, tsl], KT[:, kvh, tsl], t2[:])

            pend = None  # (kvh, tsl, ktmp) whose RoPE is not yet emitted
            for ch in range(NCH):
                tsl = slice(ch * CH, (ch + 1) * CH)
                if ch == 0:
                    hts = hts0
                else:
                    hts = p1h.tile([P, DT, CH], BF16, tag="ht")
                    for q4 in range(0, DT, 4):
                        nc.sync.dma_start(hts[:, q4:q4 + 4],
                                          ht_r[:, q4:q4 + 4, tsl])
                if ch == NCH - 1 and 2 in phases:
                    # all phase-1 DMAs are dispatched; phase-2 inputs stream
                    # behind them during the remaining phase-1 compute
                    prefetch_phase2()
                # K projection for this chunk; the RoPE rotation matmul of
                # the previous chunk issues after this projection so PE
                # never waits on the ACT PSUM->SBUF copy.
                for kvh in range(NKV):
                    kps = ps1.tile([P, CH], F32, tag="kps", bufs=2)
                    for dt in range(DT):
                        nc.tensor.matmul(
                            kps[:],
                            lhsT=wks[:, dt, kvh * P:(kvh + 1) * P],
                            rhs=hts[:, dt, :],
                            start=(dt == 0), stop=(dt == DT - 1))
                    ktmp = p1t.tile([P, CH], BF16, tag="ktmp")
                    nc.scalar.activation(ktmp[:], kps[:], AF.Copy)
                    if pend is not None:
                        k_rope(*pend)
                    pend = (kvh, tsl, ktmp)
                # V projection for the token-tiles of this chunk
                for v in range(CH // P):
                    vt = ch * (CH // P) + v
                    vps = ps1.tile([P, NKV * P], F32, tag="vps", bufs=2)
                    for dt in range(DT):
                        nc.tensor.matmul(
                            vps[:],
                            lhsT=hts[:, dt, v * P:(v + 1) * P],
                            rhs=wvs[:, dt, :],
                            start=(dt == 0), stop=(dt == DT - 1))
                    if pend is not None:
                        k_rope(*pend)
                        pend = None
                    nc.vector.tensor_copy(Vsb[:, vt, :], vps[:])

            # ---- pair exchange: export own-half K/V, AllGather, import ----
            # Own half lives at offset 0 of KT / tiles 0..7 of Vsb on both
            # cores of a pair; the gather output is ordered by core id, so
            # the import places each half at its true position.
            nc.sync.dma_start(kvd[:, 0:8, :], KT[:, :, 0:TQ])
            nc.sync.dma_start(kvd[:, 8:16, :], Vsb[:, 0:8, :])
            nc.gpsimd.collective_compute(
                "AllGather", mybir.AluOpType.bypass,
                replica_groups=[[2 * b_, 2 * b_ + 1] for b_ in range(4)],
                ins=[kvd[:]], outs=[kvg[:]],
            )
            for half in range(2):
                nc.sync.dma_start(KT[:, :, half * TQ:(half + 1) * TQ],
                                  kvg[half, :, 0:8, :])
                nc.sync.dma_start(Vsb[:, half * 8:(half + 1) * 8, :],
                                  kvg[half, :, 8:16, :])

        # ================= phase 2: Q projection + RoPE =================
        # wq.T streamed in 4-head quarters (double-buffered, quarter 0
        # prefetched during phase 1); hq resident (prefetched).
        if 2 in phases:
          if 0 not in wq_tiles:      # phase 1 skipped in a timing variant
              prefetch_phase2()
          with tc.tile_pool(name="p2t" + pfx, bufs=2) as p2t, \
             tc.tile_pool(name="ps2" + pfx, bufs=1, space="PSUM") as ps2:

            def q_rope(h, qtmp):
                rot = ps2.tile([P, TQ], F32, tag="qrot", bufs=2)
                nc.tensor.matmul(rot[:, 0:512], lhsT=pm_sb[:],
                                 rhs=qtmp[:, 0:512], start=True, stop=True)
                nc.tensor.matmul(rot[:, 512:1024], lhsT=pm_sb[:],
                                 rhs=qtmp[:, 512:1024], start=True, stop=True)
                t2 = p2t.tile([P, TQ], F32, tag="qt2")
                nc.vector.tensor_mul(t2[:], rot[:], sq_sb[:])
                nc.vector.tensor_mul(QT[:, h, :], qtmp[:], cq_sb[:])
                nc.vector.tensor_add(QT[:, h, :], QT[:, h, :], t2[:])

            pend = None  # (h, qtmp) whose RoPE is not yet emitted
            for g in range(4):               # head quarters
                if g in wq_tiles:
                    wq_sb = wq_tiles[g]
                else:
                    wq_sb = p2w.tile([P, DT, 512], BF16, tag="wq")
                    nc.sync.dma_start(wq_sb[:],
                                      wq_r[:, :, g * 512:(g + 1) * 512])
                for hh in range(4):
                    h = g * 4 + hh
                    qps = ps2.tile([P, TQ], F32, tag="qps", bufs=2)
                    for dt in range(DT):
                        for nb in range(2):
                            nc.tensor.matmul(
                                qps[:, nb * 512:(nb + 1) * 512],
                                lhsT=wq_sb[:, dt, hh * P:(hh + 1) * P],
                                rhs=hqs[:, dt, nb * 512:(nb + 1) * 512],
                                start=(dt == 0), stop=(dt == DT - 1))
                    qtmp = p2t.tile([P, TQ], BF16, tag="qtmp")
                    nc.scalar.activation(qtmp[:], qps[:], AF.Copy)
                    if pend is not None:
                        q_rope(*pend)
                    pend = (h, qtmp)
            q_rope(*pend)

        p2stack.close()

        # phase-3/4 shared tiles: ATT lives here (not in pers) so its 32 KB
        # per partition is free during phases 1-2; phase-4 weights prefetch
        # at phase-3 start so the 8 MB stream overlaps phase-3 compute.
        p4 = ctx.enter_context(tc.tile_pool(name="p4" + pfx, bufs=1))
        ATT = p4.tile([P, NH, TQ], BF16, tag="ATT")   # (d, h, q) normalized
        wo_sb = p4.tile([P, DT, D], BF16, tag="wo")
        if 4 in phases:
            wo_r = wot.rearrange("(t p) n -> p t n", p=P)
            for q2 in range(0, DT, 2):
                nc.sync.dma_start(wo_sb[:, q2:q2 + 2], wo_r[:, q2:q2 + 2, :])
        if 3 not in phases and 4 in phases:
            nc.any.memzero(ATT[:])

        # ================= phase 3: causal attention (transposed) =============
        # Local q-tile t covers global row-tile g = 2t + r; it attends to
        # k-tiles j <= 2t + 1 (the odd-parity core's diagonal; the even core
        # wastes the last one, fully masked via mask data). For k-tile j the
        # attending q suffix starts at local tile j//2.
        # Per pair p (k-tiles 2p, 2p+1): two score matmuls + exp + junction
        # mask; DVE sums the two exp'd tiles and ONE ones-matmul accumulates
        # the pair into dnp [1,512]. At group end (h, qh): ACT reciprocal of
        # dnp (bf16), PE broadcast (ones-row x recip) -> rcpb PSUM, ACT copy
        # avp->SBUF, DVE multiply -> ATT slice. PSUM: stp 2x2 + av-ring 3
        # (avp & rcpb share a 3-deep ring) + dnp 1 = 8 banks.
        if 3 in phases:
          with tc.tile_pool(name="p3t" + pfx, bufs=1) as p3t, \
             tc.tile_pool(name="ps3" + pfx, bufs=1, space="PSUM") as ps3:
            tasks = [(h, qh, p)
                     for h in range(NH)
                     for qh in range(2)
                     for p in range(4 * (qh + 1))]
            # denominator collect: one tile set per qh half, each based at
            # partition 0 (sliced/base-32 operands misbehave on PE/DVE)
            selr_sb = p3t.tile([NH, 2 * NH * P], BF16, tag="selr")
            nc.sync.dma_start(selr_sb[:], selr[:])
            DC = [p3t.tile([NH, 512], F32, tag=f"DC{q_}", name=f"DC{q_}")
                  for q_ in range(2)]
            DCs = [p3t.tile([NH, 512], F32, tag=f"DCs{q_}", name=f"DCs{q_}")
                   for q_ in range(2)]
            DCx = [p3t.tile([NH, 512], F32, tag=f"DCx{q_}", name=f"DCx{q_}")
                   for q_ in range(2)]
            DCb = [p3t.tile([NH, 512], BF16, tag=f"DCb{q_}", name=f"DCb{q_}")
                   for q_ in range(2)]
            state = {}           # (h, qh) -> (avp, dnp)
            prev = None          # (h, qh, p, pt)
            dn_q = []            # deferred dn matmuls: [count, entry...]
            norm_q = []          # deferred drains: [count, h, qh, avp, dnp]

            def emit_dn(e):
                _, dnp, pts, pqs, first, last, ph, pqh, avp = e
                nc.tensor.matmul(dnp[:, pqs:512], lhsT=ones_sb[:],
                                 rhs=pts[:, pqs:512],
                                 start=first, stop=last)
                if last:
                    norm_q.append([1, ph, pqh, avp, dnp])

            drained = [0, 0]     # groups drained per qh half
            bcast_q = []         # pending (row, h, qh) normalize chains
            acc_first = {}       # (h,qh) -> first pair-sum (qh=1 merge)
            acc_state = {}       # (h,qh) -> running p0..p4 accumulator

            def emit_drain(e):
                # group drain: unnormalized attn -> ATT (bf16), dn -> DC row.
                # Both are cheap DVE copies; reciprocals are batched per qh
                # half (a [1,512] DVE reciprocal is 3.3us and would stall
                # the DVE FIFO once per group).
                _, ph, pqh, avp, dnp = e
                pqbase = pqh * 512
                nc.vector.tensor_copy(ATT[:, ph, pqbase:pqbase + 512], avp[:])
                dns = p3t.tile([1, 512], F32, tag="dns", bufs=2)
                nc.vector.tensor_copy(dns[:], dnp[:])
                nc.sync.dma_start(DC[pqh][ph:ph + 1, :], dns[:])
                drained[pqh] += 1
                if drained[pqh] == NH:
                    # all groups of this half are in DC: batched reciprocal,
                    # then one bcast+multiply chain per group, spread one per
                    # task so the PE never bursts on the rcpb ring
                    nc.vector.reciprocal_approx_accurate(DCs[pqh][:],
                                                         DC[pqh][:],
                                                         DCx[pqh][:])
                    nc.vector.tensor_copy(DCb[pqh][:], DCs[pqh][:])
                    bcast_q.extend((pqh * NH + h2, h2, pqh)
                                   for h2 in range(NH))

            def emit_bcast(idx, bh, bqh):
                qbase = bqh * 512
                rcpb = ps3.tile([P, 512], F32, tag="av", bufs=3,
                                name=f"rcpb{pfx}_{bh}_{bqh}")
                nc.tensor.matmul(rcpb[:],
                                 lhsT=selr_sb[:, idx * P:(idx + 1) * P],
                                 rhs=DCb[bqh][:],
                                 start=True, stop=True)
                nc.vector.tensor_mul(ATT[:, bh, qbase:qbase + 512],
                                     ATT[:, bh, qbase:qbase + 512],
                                     rcpb[:])

            for tsk in tasks + [None] * 4:
                if tsk is not None:
                    h, qh, p = tsk
                    kvh = h // 4
                    qbase = qh * 512
                    qs = max(0, p - 4 * qh) * P
                    # stp allocated first so its ring lands on the earliest-
                    # released phase-2 PSUM zones (no wait on the last head's
                    # rope tail)
                    stp = ps3.tile([P, 2, 512], F32, tag="st", bufs=2)
                    if p == 0:
                        avp_new = ps3.tile([P, 512], F32, tag="av", bufs=3,
                                           name=f"avp{pfx}_{h}_{qh}")
                        dnp_new = ps3.tile([1, 512], F32, tag="dn", bufs=1,
                                           name=f"dnp{pfx}_{h}_{qh}")
                        state[(h, qh)] = (avp_new, dnp_new)
                    for jj in range(2):
                        j = 2 * p + jj
                        nc.tensor.matmul(
                            stp[:, jj, qs:512],
                            lhsT=KT[:, kvh, j * P:(j + 1) * P],
                            rhs=QT[:, h, qbase + qs:qbase + 512],
                            start=True, stop=True)
                    pt = p3t.tile([P, 2, 512], BF16, tag="pt", bufs=3)
                    nc.scalar.activation(pt[:, :, qs:512],
                                         stp[:, :, qs:512], AF.Exp)
                    if p >= 4 * qh:      # diagonal junction: causal mask
                        nc.vector.tensor_mul(pt[:, :, qs:qs + P],
                                             pt[:, :, qs:qs + P], mask_sb[:])
                    ptsum = p3t.tile([P, 512], BF16, tag="pts", bufs=4)
                    nc.vector.tensor_add(ptsum[:, qs:512], pt[:, 0, qs:512],
                                         pt[:, 1, qs:512])
                    # qh=1 pairs p0..p4 share qs=0: accumulate their pair
                    # sums on DVE so ONE dn matmul covers all five pairs
                    if qh == 1 and p <= 4:
                        if p == 0:
                            acc_first[(h, qh)] = ptsum
                        elif p == 1:
                            ptacc = p3t.tile([P, 512], BF16, tag="pta",
                                             bufs=2)
                            acc_state[(h, qh)] = ptacc
                            nc.vector.tensor_add(
                                ptacc[:], acc_first.pop((h, qh))[:],
                                ptsum[:])
                        else:
                            ptacc = acc_state[(h, qh)]
                            nc.vector.tensor_add(ptacc[:], ptacc[:],
                                                 ptsum[:])
                else:
                    pt = ptsum = None
                # deferred work.  norm before dn: a group's reciprocal must
                # read dnp (bufs=1) before the next group's first dn matmul
                # resets it.
                for e in norm_q:
                    e[0] -= 1
                for e in dn_q:
                    e[0] -= 1
                while norm_q and (tsk is None or norm_q[0][0] <= 0):
                    emit_drain(norm_q.pop(0))
                while dn_q and (tsk is None or dn_q[0][0] <= 0):
                    emit_dn(dn_q.pop(0))
                npop = len(bcast_q) if tsk is None else 2
                for _ in range(min(npop, len(bcast_q))):
                    emit_bcast(*bcast_q.pop(0))
                if prev is not None:
                    ph, pqh, pp, ppt, pps = prev
                    pkvh = ph // 4
                    E = 8 * (pqh + 1)
                    pqs = max(0, pp - 4 * pqh) * P
                    avp, dnp = state[(ph, pqh)]
                    for jj in range(2):
                        j = 2 * pp + jj
                        nc.tensor.matmul(
                            avp[:, pqs:512],
                            lhsT=Vsb[:, j, pkvh * P:(pkvh + 1) * P],
                            rhs=ppt[:, jj, pqs:512],
                            start=(j == 0), stop=(j == E - 1))
                    if pqh == 1 and pp <= 4:
                        if pp == 4:   # merged dn for pairs 0..4
                            dn_q.append([2, dnp, acc_state.pop((ph, pqh)),
                                         0, True, False, ph, pqh, avp])
                    else:
                        dn_q.append([2, dnp, pps, pqs, pp == 0,
                                     pp == E // 2 - 1, ph, pqh, avp])
                    if pp == E // 2 - 1:
                        del state[(ph, pqh)]
                prev = (h, qh, p, pt, ptsum) if tsk is not None else None



        # ================= phase 4: output projection =================
        if 4 in phases:
          with tc.tile_pool(name="p4o" + pfx, bufs=1) as p4o, \
             tc.tile_pool(name="ps4" + pfx, bufs=1, space="PSUM") as ps4:
            last_osb = None
            for tt in range(TQ // P):
                for cb in range(2):
                    ops = ps4.tile([P, 1024], F32, tag="ops", bufs=2)
                    for htile in range(NH):
                        for nb in range(2):
                            nc.tensor.matmul(
                                ops[:, nb * 512:(nb + 1) * 512],
                                lhsT=ATT[:, htile, tt * P:(tt + 1) * P],
                                rhs=wo_sb[:, htile,
                                          cb * 1024 + nb * 512:cb * 1024 + (nb + 1) * 512],
                                start=(htile == 0), stop=(htile == NH - 1))
                    osb = p4o.tile([P, 1024], F32, tag="osb", bufs=3)
                    nc.scalar.activation(osb[:], ops[:], AF.Copy)
                    nc.sync.dma_start(
                        out[tt * P:(tt + 1) * P, cb * 1024:(cb + 1) * 1024], osb[:])
                    last_osb = osb

            # timing-chain output: tock = tick, ordered after the last store
            tock_sb = p4o.tile([1, 8], F32, tag="tock")
            nc.vector.tensor_tensor(tock_sb[:], tick_sb[:], last_osb[0:1, 0:8],
                                    mybir.AluOpType.bypass)
            nc.sync.dma_start(tock[:], tock_sb[:])


# ---------------------------------------------------------------------------
# host-side wrapper
# ---------------------------------------------------------------------------

_NC = None


def _get_nc():
    global _NC
    if _NC is None:
        _NC = build_program()
    return _NC


def make_in_maps(hidden_states, cos, sin, wq, wk, wv, wo):
    """Build the 8 per-core input dicts (host-side sharding/layout prep)."""
    scale = np.float32(1.0 / np.sqrt(P))
    wqt = np.ascontiguousarray(wq.T).astype(NPBF16)
    wkt = np.ascontiguousarray(wk.T).astype(NPBF16)
    wvt = np.ascontiguousarray(wv.T).astype(NPBF16)
    wot = np.ascontiguousarray(wo.T).astype(NPBF16)
    pmat = np.zeros((P, P), np.float32)
    for m in range(64):
        pmat[m + 64, m] = -1.0      # out[m] = -in[m+64]
        pmat[m, m + 64] = 1.0       # out[m+64] = in[m]
    onesc = np.ones((P, 1), NPBF16)
    selr = np.zeros((NH, 2 * NH * P), NPBF16)
    for idx in range(2 * NH):
        selr[idx % NH, idx * P:(idx + 1) * P] = 1.0
    tri = (np.arange(P)[:, None] <= np.arange(P)[None, :])  # [k, q]: k <= q

    in_maps = []
    for c in range(N_CORES):
        b, r = c // 2, c % 2
        hb = np.asarray(hidden_states[b])                   # [T, D] f32
        own = hb.reshape(T // P, P, D)[r::2].reshape(TQ, D)
        cosb = np.asarray(cos[b])                           # [T, 128]
        sinb = np.asarray(sin[b])
        cow = cosb.reshape(T // P, P, P)[r::2].reshape(TQ, P)
        sow = sinb.reshape(T // P, P, P)[r::2].reshape(TQ, P)
        maskab = np.empty((P, 2, P), np.float32)
        if r == 0:
            maskab[:, 0, :] = tri       # even j is the diagonal
            maskab[:, 1, :] = 0.0       # odd j fully masked (waste tile)
        else:
            maskab[:, 0, :] = 1.0       # even j unmasked
            maskab[:, 1, :] = tri       # odd j is the diagonal
        hsl = slice(r * TQ, (r + 1) * TQ)   # own contiguous token half (K/V)
        in_maps.append({
            "hth": np.ascontiguousarray(hb[hsl].T).astype(NPBF16),
            "hq": np.ascontiguousarray(own.T).astype(NPBF16),
            "wqt": wqt, "wkt": wkt, "wvt": wvt, "wot": wot,
            "cq": np.ascontiguousarray(cow.T) * scale,
            "sq": np.ascontiguousarray(sow.T) * scale,
            "ck": np.ascontiguousarray(cosb[hsl].T),
            "sk": np.ascontiguousarray(sinb[hsl].T),
            "maskab": maskab.astype(NPBF16),
            "pm": pmat.astype(NPBF16),
            "onesc": onesc,
            "selr": selr,
            "tick": np.zeros((1, 8), np.float32),
        })
    return in_maps


def assemble_output(results):
    out = np.empty((4, T, D), np.float32)
    for c in range(N_CORES):
        b, r = c // 2, c % 2
        out[b].reshape(T // P, P, D)[r::2] = results[c]["out"].reshape(TQ // P, P, D)
    return out


def kernel(hidden_states, cos, sin, wq, wk, wv, wo):
    nc = _get_nc()
    in_maps = make_in_maps(hidden_states, cos, sin, wq, wk, wv, wo)
    res = run_bass_kernel_spmd(nc, in_maps, list(range(N_CORES)))
    return assemble_output(res.results)


if __name__ == "__main__":
    rng = np.random.default_rng(0)
    args = {
        "hidden_states": rng.standard_normal((4, T, D), np.float32),
        "cos": rng.random((4, T, P), np.float32),
        "sin": rng.random((4, T, P), np.float32),
        "wq": rng.standard_normal((NH * P, D), np.float32) / np.sqrt(D),
        "wk": rng.standard_normal((NKV * P, D), np.float32) / np.sqrt(D),
        "wv": rng.standard_normal((NKV * P, D), np.float32) / np.sqrt(D),
        "wo": rng.standard_normal((D, NH * P), np.float32) / np.sqrt(D),
    }
    o = kernel(**args)
    print("ran:", o.shape, o.dtype, np.abs(o).max())
